# revision 21
# baseline (speedup 1.0000x reference)
"""Local (sliding-window) MQA attention block on 8 Trainium2 NeuronCores.

Sharding: data-parallel over batch (4) x sequence-parallel over query halves
(2) = 8 cores. Each core computes 1024 query rows of one batch against a
2048-row key halo (window=1024), all 16 query heads, with the single shared
KV head replicated. Outputs are disjoint row-slices of the final projection,
so no cross-core collectives are needed.

v2 layout: all PE operands in bf16 (fast weight load + half the HBM
traffic); x is loaded once (query half kept SBUF-resident and reused by the
q projection); attention S/exp/PV/den restricted to the structurally
non-masked column ranges per key slot with 128x128 triangular masks; the
softmax reciprocal uses the fast approx DVE op; the output projection is
interleaved per query block so the tensor engine absorbs the ACT-bound exp
stretches.

Device kernel phases (per core, identical SPMD program; data differs):
  A1) k/v projection of the query-half keys (also captures x^T into SBUF).
  B)  q projection for all 16 heads from the resident x^T, RoPE fused.
  A0) k/v projection of the remaining halo keys (DMA overlapped with B).
  C)  blocked attention in transposed layout, interleaved per query block
      with D) the output projection + bias.
"""
import sys

for _p in ("/opt/trn_rl_repo",):
    if _p not in sys.path:
        sys.path.insert(0, _p)

import ml_dtypes
import numpy as np

import concourse.bass as bass
import concourse.bacc as bacc
import concourse.tile as tile
import concourse.mybir as mybir
from concourse.bass_utils import run_bass_kernel_spmd

F32 = mybir.dt.float32
BF16 = mybir.dt.bfloat16
NPBF = ml_dtypes.bfloat16
EXP = mybir.ActivationFunctionType.Exp

B, T, W = 4, 2048, 2048
NH, HD = 16, 128
WIN = 1024
QL = 1024          # query rows per core
KB = 2048          # key-halo rows per core
QBS = 512          # query block (moving free dim)
NQB = QL // QBS    # 2 query blocks per core
SLOTS = (WIN + QBS) // 128  # 12 key slots of 128 per query block
NEG = -1.0e9
SCALE = HD ** -0.5
MAX_WAVELENGTH = 10000.0
NW = W // 128      # 16 width chunks

# Attention slot schedule: interior slots first (full query range, so the
# PSUM accumulations for den/PV start with a full write), then the partial
# window-edge (0-3) and causal-diagonal (8-11) slots with restricted column
# ranges. MCOL maps masked slot -> triangle index in the mask tensor.
ORDER = [4, 5, 6, 7, 0, 1, 2, 3, 8, 9, 10, 11]
MCOL = {0: 0, 1: 1, 2: 2, 3: 3, 8: 4, 9: 5, 10: 6, 11: 7}


def _slot_cols(k):
    """(c0, c1, mc0) query-column range computed for slot k and the start of
    its 128-wide mask triangle (None if unmasked)."""
    if 4 <= k <= 7:
        return 0, QBS, None
    if k <= 3:                 # window left edge: cols [0, 128*(k+1))
        return 0, 128 * (k + 1), 128 * k
    c0 = 128 * (k - 8)         # causal diagonal: cols [128*(k-8), QBS)
    return c0, QBS, c0


_COMPILED = None


def _rope_tables(pos):
    """pos: [n] int -> (cmul, smul) [128, n] f32 such that
    rope(x)[d] = x[d]*cmul[d] + x[shuf(d)]*smul[d], shuf(d)=d^32 for d<64."""
    half = 32
    inv_freq = MAX_WAVELENGTH ** (-(2.0 * np.arange(half, dtype=np.float64)) / 64.0)
    ang = pos.astype(np.float64)[None, :] * inv_freq[:, None]   # [32, n]
    sin, cos = np.sin(ang), np.cos(ang)
    n = pos.shape[0]
    cmul = np.ones((HD, n), dtype=np.float64)
    smul = np.zeros((HD, n), dtype=np.float64)
    cmul[0:32] = cos
    cmul[32:64] = cos
    smul[0:32] = -sin
    smul[32:64] = sin
    return cmul.astype(np.float32), smul.astype(np.float32)


def _emit_rope(nc, pool, dst, src_ps, cmul, smul, n):
    """dst[0:64] = src[0:64]*c[0:64] + shuf(src)[0:64]*s[0:64]; dst[64:128]=src.
    dst: SBUF bf16 AP [128, n]; src_ps: PSUM f32 AP [128, n]; cmul/smul f32.
    The partition shuffle is folded into the smul multiplies via offset
    partition APs; only the pass-through copy runs on ScalarE."""
    t1 = pool.tile([64, n], F32, tag="rope_t1", bufs=2)
    t2 = pool.tile([64, n], F32, tag="rope_t2", bufs=2)
    nc.vector.tensor_mul(t1[:, :], src_ps[0:64, :], cmul[0:64, :])
    nc.vector.tensor_mul(t2[0:32, :], src_ps[32:64, :], smul[0:32, :])
    nc.vector.tensor_mul(t2[32:64, :], src_ps[0:32, :], smul[32:64, :])
    nc.vector.tensor_add(dst[0:64, :], t1[:, :], t2[:, :])
    nc.scalar.copy(out=dst[64:128, :], in_=src_ps[64:128, :])


def _build_program():
    nc = bacc.Bacc("TRN2", target_bir_lowering=False, debug=False)

    xt = nc.dram_tensor("xt", [2, W, KB // 2], BF16, kind="ExternalInput")
    wqt = nc.dram_tensor("wqt", [W, W], BF16, kind="ExternalInput")
    wkt = nc.dram_tensor("wkt", [W, HD], BF16, kind="ExternalInput")
    wvt = nc.dram_tensor("wvt", [W, HD], BF16, kind="ExternalInput")
    wot = nc.dram_tensor("wot", [W, W], BF16, kind="ExternalInput")
    bias = nc.dram_tensor("bias", [W], F32, kind="ExternalInput")
    cq_d = nc.dram_tensor("cq", [HD, QL], BF16, kind="ExternalInput")
    sq_d = nc.dram_tensor("sq", [HD, QL], BF16, kind="ExternalInput")
    ck_d = nc.dram_tensor("ck", [HD, KB], BF16, kind="ExternalInput")
    sk_d = nc.dram_tensor("sk", [HD, KB], BF16, kind="ExternalInput")
    masktri_d = nc.dram_tensor("masktri", [128, NQB, 8, 128], BF16,
                               kind="ExternalInput")
    ident_d = nc.dram_tensor("ident", [128, 128], BF16, kind="ExternalInput")
    ones_d = nc.dram_tensor("ones", [128, 1], BF16, kind="ExternalInput")
    padb_d = nc.dram_tensor("padb", [128, KB // 128], F32, kind="ExternalInput")
    out = nc.dram_tensor("out", [QL, W], F32, kind="ExternalOutput")

    with tile.TileContext(nc) as tc:
        with tc.tile_pool(name="persist", bufs=1) as pp:
            kTq = [pp.tile([HD, 512], BF16, tag="kT", bufs=4,
                           name=f"kT{j}") for j in range(4)]  # rope'd k^T
            vq = [pp.tile([128, 512], BF16, tag="v", bufs=4,
                          name=f"v{j}") for j in range(4)]    # natural v
            qT_w = [pp.tile([HD, 4, QL], BF16, tag="qT", bufs=4,
                            name=f"qT{w}") for w in range(4)]
            masks = pp.tile([128, NQB, 8, 128], BF16, tag="masks")
            ident = pp.tile([128, 128], BF16, tag="ident")
            ones_sb = pp.tile([128, 1], BF16, tag="ones")
            bias_bc = pp.tile([128, W], F32, tag="biasbc")
            padb = pp.tile([128, KB // 128], F32, tag="padb")

            # ---------- Phases A/B: projections ----------
            with tc.tile_pool(name="ab", bufs=1) as ab, \
                 tc.tile_pool(name="pb", bufs=3) as pb:
                # resident x^T query rows, one tile per width chunk so each
                # kv/q matmul depends only on its own chunk's DMA
                xq = [ab.tile([128, QL], BF16, tag="xq", bufs=NW,
                              name=f"xq{w}") for w in range(NW)]
                wk_sb = ab.tile([128, NW, HD], BF16, tag="wk")
                wv_sb = ab.tile([128, NW, HD], BF16, tag="wv")
                cq = ab.tile([HD, QL], BF16, tag="cq")
                sq = ab.tile([HD, QL], BF16, tag="sq")
                ck = ab.tile([HD, KB], BF16, tag="ck")
                sk = ab.tile([HD, KB], BF16, tag="sk")
                nc.scalar.dma_start(
                    out=wk_sb[:, :, :],
                    in_=wkt.ap().rearrange("(c p) h -> p c h", p=128))
                nc.scalar.dma_start(
                    out=wv_sb[:, :, :],
                    in_=wvt.ap().rearrange("(c p) h -> p c h", p=128))

                # wq wave tiles; waves 0/1 prefetched before phase A's rope
                # work occupies the scalar queue.
                wq_tiles = [pb.tile([128, NW, 512], BF16, tag="wqw", bufs=2,
                                    name=f"wqw{w}") for w in range(4)]

                def dma_wq(wave):
                    for dm in range(4):
                        nc.scalar.dma_start(
                            out=wq_tiles[wave][:, 4 * dm:4 * (dm + 1), :],
                            in_=wqt[512 * dm:512 * (dm + 1),
                                    128 * 4 * wave:128 * 4 * (wave + 1)]
                            .rearrange("(c p) h -> p c h", p=128))
                dma_wq(0)

                nc.gpsimd.dma_start(out=ck[:, :], in_=ck_d[:, :])
                nc.gpsimd.dma_start(out=sk[:, :], in_=sk_d[:, :])
                nc.gpsimd.dma_start(out=ident[:, :], in_=ident_d[:, :])
                nc.gpsimd.dma_start(out=cq[:, :], in_=cq_d[:, :])
                nc.gpsimd.dma_start(out=sq[:, :], in_=sq_d[:, :])
                nc.gpsimd.dma_start(out=ones_sb[:, :], in_=ones_d[:, :])

                def emit_kv_half(sq2, pa, paps, xt_tile):
                    """kv projection of halo cols [1024*sq2, 1024*(sq2+1)).
                    xt_tile(wc) -> SBUF AP [128, 1024] holding that x^T chunk
                    (also responsible for issuing its DMA)."""
                    kt_ps = [paps.tile([HD, 512], F32, tag="kt_ps",
                                       name="kt_ps") for _ in range(2)]
                    vt_ps = [paps.tile([HD, 512], F32, tag="vt_ps",
                                       name="vt_ps") for _ in range(2)]
                    for wc in range(NW):
                        xch = xt_tile(wc)
                        for hf in range(2):
                            nc.tensor.matmul(
                                out=kt_ps[hf][:, :], lhsT=wk_sb[:, wc, :],
                                rhs=xch[:, QBS * hf:QBS * (hf + 1)],
                                start=(wc == 0), stop=(wc == NW - 1))
                            nc.tensor.matmul(
                                out=vt_ps[hf][:, :], lhsT=wv_sb[:, wc, :],
                                rhs=xch[:, QBS * hf:QBS * (hf + 1)],
                                start=(wc == 0), stop=(wc == NW - 1))
                    for hf in ((1, 0) if sq2 == 0 else (0, 1)):
                        sq4 = 2 * sq2 + hf
                        cols = slice(512 * sq4, 512 * (sq4 + 1))
                        _emit_rope(nc, pa, kTq[sq4][:, :], kt_ps[hf][:, :],
                                   ck[:, cols], sk[:, cols], 512)
                        # v: copy PSUM->SBUF then PE-transpose 128-blocks
                        vt_sb = pa.tile([HD, 512], BF16, tag="vt_sb")
                        nc.vector.tensor_copy(out=vt_sb[:, :],
                                              in_=vt_ps[hf][:, :])
                        for j in range(4):
                            vps2 = paps.tile([128, 128], BF16, tag="vT2")
                            nc.tensor.transpose(
                                vps2[:, :], vt_sb[:, 128 * j:128 * (j + 1)],
                                ident[:, :])
                            nc.vector.tensor_copy(
                                out=vq[sq4][:, 128 * j:128 * (j + 1)],
                                in_=vps2[:, :])

                # ----- A1: query-half keys; captures xq -----
                with tc.tile_pool(name="pa1", bufs=3) as pa, \
                     tc.tile_pool(name="pa1_ps", bufs=2, space="PSUM") as paps:
                    def xq_tile(wc):
                        nc.sync.dma_start(
                            out=xq[wc][:, :],
                            in_=xt[0, 128 * wc:128 * (wc + 1), :])
                        return xq[wc][:, :]
                    emit_kv_half(1, pa, paps, xq_tile)

                # ----- A0: halo keys, streamed right behind the xq DMAs
                # (the x stream saturates HBM while the PE does kv-proj) -----
                with tc.tile_pool(name="pa0", bufs=3) as pa0s, \
                     tc.tile_pool(name="pa0_ps", bufs=2, space="PSUM") as paps0:
                    def xh_tile(wc):
                        t = pa0s.tile([128, 1024], BF16, tag="xh")
                        nc.sync.dma_start(
                            out=t[:, :], in_=xt[1, 128 * wc:128 * (wc + 1), :])
                        return t[:, :]
                    emit_kv_half(0, pa0s, paps0, xh_tile)

                dma_wq(1)
                # ----- B: q projection, all 16 heads -----
                with tc.tile_pool(name="pb_ps", bufs=8, space="PSUM") as pbps:
                    for wave in range(4):
                        q_ps = [[pbps.tile([HD, QBS], F32, tag="q_ps",
                                           name="q_ps")
                                 for _ in range(2)] for _ in range(4)]
                        h0 = wave * 4   # first head this wave
                        wq_w = wq_tiles[wave]
                        for wc in range(NW):
                            for j4 in range(4):
                                for qh in range(2):
                                    nc.tensor.matmul(
                                        out=q_ps[j4][qh][:, :],
                                        lhsT=wq_w[:, wc,
                                                  128 * j4:128 * (j4 + 1)],
                                        rhs=xq[wc][:, QBS * qh:QBS * (qh + 1)],
                                        start=(wc == 0),
                                        stop=(wc == NW - 1))
                        if wave < 2:
                            dma_wq(wave + 2)
                        for j4 in range(4):
                            head = h0 + j4
                            for qh in range(2):
                                _emit_rope(
                                    nc, pb,
                                    qT_w[wave][:, j4, QBS * qh:QBS * (qh + 1)],
                                    q_ps[j4][qh][:, :],
                                    cq[:, QBS * qh:QBS * (qh + 1)],
                                    sq[:, QBS * qh:QBS * (qh + 1)], QBS)


            # ---------- Phases C+D interleaved per query block ----------
            GS = 2  # slots per pipeline group
            NG = SLOTS // GS
            LA = 3  # acc groups trail S/exp groups by LA
            with tc.tile_pool(name="penc", bufs=1) as penc, \
                 tc.tile_pool(name="pd", bufs=2) as pd, \
                 tc.tile_pool(name="pdo", bufs=3) as pdo:
                encU = penc.tile([HD, NH, QL], BF16, tag="encU")
                nc.gpsimd.dma_start(out=padb[:, :], in_=padb_d[:, :])
                nc.gpsimd.dma_start(out=masks[:, :, :, :],
                                    in_=masktri_d[:, :, :, :])
                b_ap = bias.ap()
                nc.gpsimd.dma_start(out=bias_bc[:, :], in_=bass.AP(
                    tensor=b_ap.tensor, offset=b_ap.offset,
                    ap=[[0, 128]] + list(b_ap.ap)))

                def emit_d_tile(i, oc, tsub, wot_sb, dpool):
                    o_ps = dpool.tile([128, 512], F32, tag="o_ps")
                    for n in range(NH):
                        nc.tensor.matmul(
                            out=o_ps[:, :],
                            lhsT=encU[:, n, 128 * tsub:128 * (tsub + 1)],
                            rhs=wot_sb[:, n, :],
                            start=(n == 0), stop=(n == NH - 1))
                    o_sb = pdo.tile([128, 512], F32, tag="o_sb")
                    nc.vector.tensor_add(
                        o_sb[:, :], o_ps[:, :],
                        bias_bc[:, 512 * oc:512 * (oc + 1)])
                    nc.sync.dma_start(
                        out=out[128 * tsub:128 * (tsub + 1),
                                512 * oc:512 * (oc + 1)],
                        in_=o_sb[:, :])

                def dma_wot(oc):
                    wot_sb = pd.tile([128, NW, 512], BF16, tag="wot")
                    for dm in range(4):
                        nc.gpsimd.dma_start(
                            out=wot_sb[:, 4 * dm:4 * (dm + 1), :],
                            in_=wot[512 * dm:512 * (dm + 1),
                                    512 * oc:512 * (oc + 1)]
                            .rearrange("(c p) h -> p c h", p=128))
                    return wot_sb

                CP = {}  # C-phase pools, bound once the inner scope opens

                def emit_head(i, head):
                    pc, pe_t, pcs, pca, pcd = (CP['pc'], CP['et'], CP['pcs'],
                                               CP['pca'], CP['pcd'])
                    enc_ps = pca.tile([HD, QBS], F32, tag="enc_ps")
                    den_ps = pcd.tile([1, QBS], F32, tag="den_ps")
                    ets = [None] * SLOTS
                    eint = [None]
                    qrow = qT_w[head // 4][:, head % 4, :]

                    def emit_s_group(g):
                        sps = []
                        for kk in range(GS):
                            k = ORDER[GS * g + kk]
                            c0, c1, mc0 = _slot_cols(k)
                            nc_ = c1 - c0
                            s_ps = pcs.tile([128, QBS], F32, tag="s_ps")
                            kc = 512 * i + 128 * k
                            nc.tensor.matmul(
                                out=s_ps[:, 0:nc_],
                                lhsT=kTq[kc // 512][:, kc % 512:kc % 512 + 128],
                                rhs=qrow[:, QBS * i + c0:QBS * i + c1],
                                start=True, stop=(mc0 is None))
                            if mc0 is not None:
                                # += triangular mask on PE
                                t0 = mc0 - c0
                                nc.tensor.matmul(
                                    out=s_ps[:, t0:t0 + 128],
                                    lhsT=ident[:, :],
                                    rhs=masks[:, i, MCOL[k], :],
                                    start=False, stop=True)
                            sps.append((s_ps, nc_))
                        for kk in range(GS):
                            k = ORDER[GS * g + kk]
                            blk = 4 * i + k
                            s_ps, nc_ = sps[kk]
                            et = pe_t.tile([128, QBS], BF16, tag="et")
                            nc.scalar.activation(
                                out=et[:, 0:nc_], in_=s_ps[:, 0:nc_],
                                func=EXP,
                                bias=padb[:, blk:blk + 1])
                            ets[k] = et
                        if g == 1:
                            # presum the 4 full interior slots on DVE so den
                            # needs one full-range matmul instead of four
                            e45 = pe_t.tile([128, QBS], BF16, tag="eA")
                            e67 = pe_t.tile([128, QBS], BF16, tag="eB")
                            es = pe_t.tile([128, QBS], BF16, tag="eS")
                            nc.vector.tensor_add(e45[:, :], ets[4][:, :],
                                                 ets[5][:, :])
                            nc.vector.tensor_add(e67[:, :], ets[6][:, :],
                                                 ets[7][:, :])
                            nc.vector.tensor_add(es[:, :], e45[:, :],
                                                 e67[:, :])
                            eint[0] = es

                    def emit_acc_group(g):
                        if g == 1:
                            # den of all four interior slots via the presum
                            nc.tensor.matmul(
                                out=den_ps[:, :], lhsT=ones_sb[:, :],
                                rhs=eint[0][:, :], start=True, stop=False)
                        for kk in range(GS):
                            k = ORDER[GS * g + kk]
                            c0, c1, _ = _slot_cols(k)
                            if not (4 <= k <= 7):
                                nc.tensor.matmul(
                                    out=den_ps[:, c0:c1],
                                    lhsT=ones_sb[:, :],
                                    rhs=ets[k][:, 0:c1 - c0],
                                    start=False,
                                    stop=(g == NG - 1 and kk == GS - 1))
                        for kk in range(GS):
                            k = ORDER[GS * g + kk]
                            c0, c1, _ = _slot_cols(k)
                            blk = 4 * i + k
                            nc.tensor.matmul(
                                out=enc_ps[:, c0:c1],
                                lhsT=vq[blk // 4][:, 128 * (blk % 4):
                                                  128 * (blk % 4 + 1)],
                                rhs=ets[k][:, 0:c1 - c0],
                                start=(g == 0 and kk == 0),
                                stop=(g == NG - 1 and kk == GS - 1))

                    # software pipeline: S groups LA ahead of acc groups
                    for g in range(LA):
                        emit_s_group(g)
                    for g in range(LA, NG):
                        emit_s_group(g)
                        emit_acc_group(g - LA)
                    for g in range(NG - LA, NG):
                        emit_acc_group(g)

                    den_sb = pc.tile([1, QBS], F32, tag="den_sb")
                    nc.vector.reciprocal_approx_fast(
                        out=den_sb[:, :], in_=den_ps[:, :])
                    den_bc = pc.tile([128, QBS], F32, tag="den_bc")
                    nc.gpsimd.partition_broadcast(
                        den_bc[:, :], den_sb[:, :])
                    nc.vector.tensor_mul(
                        encU[:, head, QBS * i:QBS * (i + 1)],
                        enc_ps[:, :], den_bc[:, :])

                with tc.tile_pool(name="pc", bufs=3) as pc, \
                     tc.tile_pool(name="et", bufs=12) as pe_t, \
                     tc.tile_pool(name="pc_s", bufs=4, space="PSUM") as pcs, \
                     tc.tile_pool(name="pc_a", bufs=2, space="PSUM") as pca, \
                     tc.tile_pool(name="pc_d", bufs=1, space="PSUM") as pcd, \
                     tc.tile_pool(name="pd_ps", bufs=1, space="PSUM") as pdps:
                    CP.update(pc=pc, et=pe_t, pcs=pcs, pca=pca, pcd=pcd)
                    # C(i=0)
                    for head in range(NH):
                        emit_head(0, head)
                    # C(i=1) braided with D(i=0): one D tile after each head
                    wot_sb = None
                    for head in range(NH):
                        if head % 4 == 0:
                            wot_sb = dma_wot(head // 4)
                        emit_head(1, head)
                        emit_d_tile(0, head // 4, head % 4, wot_sb, pdps)
                # D(i=1) with its own triple-buffered PSUM accumulators
                with tc.tile_pool(name="pd2_ps", bufs=3, space="PSUM") as pdps2:
                    for oc in range(4):
                        wot_sb = dma_wot(oc)
                        for tsub in range(4, 8):
                            emit_d_tile(1, oc, tsub, wot_sb, pdps2)

    nc.compile()
    return nc


def _get_program():
    global _COMPILED
    if _COMPILED is None:
        _COMPILED = _build_program()
    return _COMPILED


def _prep_core_inputs(x, segment_pos, attention_mask, shared):
    """Per-core input dicts. Core c: batch c//2, query half c%2."""
    segment_pos = np.asarray(segment_pos)
    attention_mask = np.asarray(attention_mask)
    in_maps = []
    for c in range(8):
        b, h = c // 2, c % 2
        key_start = QL * h - WIN
        # halo buffer rows [key_start, key_start + KB) of batch b, zero-padded
        kb = np.zeros((KB, W), dtype=np.float32)
        lo = max(0, -key_start)
        kb[lo:] = x[b, key_start + lo:key_start + KB]
        xtT = kb.T.astype(NPBF)
        xt = np.ascontiguousarray(
            np.stack([xtT[:, WIN:], xtT[:, :WIN]]))  # [2, W, 1024]

        g_q = QL * h + np.arange(QL)                      # global query rows
        g_k = key_start + np.arange(KB)                   # global key rows
        pos_q = segment_pos[g_q]
        pos_k = np.where((g_k >= 0) & (g_k < T), segment_pos[np.clip(g_k, 0, T - 1)], 0)
        cq, sq = (t.astype(NPBF) for t in _rope_tables(pos_q))
        ck, sk = (t.astype(NPBF) for t in _rope_tables(pos_k))

        # Additive mask per (query block i, slot k) in S^T layout [ds, dt],
        # then verified against the restricted-column structure and reduced
        # to 128x128 triangles. Out-of-range (halo padding) rows get NO
        # penalty (excluded via padb in the exp bias; v rows are 0).
        tri = np.zeros((NQB, 8, 128, 128), dtype=np.float32)
        for i in range(NQB):
            t_glob = g_q[QBS * i:QBS * (i + 1)]           # [dt=512]
            for k in range(SLOTS):
                r = QBS * i + 128 * k + np.arange(128)    # halo rows [ds]
                s_glob = key_start + r
                ok = (s_glob >= 0) & (s_glob < T)
                m = attention_mask[t_glob[None, :].repeat(128, 0),
                                   np.clip(s_glob, 0, T - 1)[:, None]]
                pen = ok[:, None] & ~m                    # [128, 512] penalty
                if 4 <= k <= 7:
                    if pen.any():
                        raise ValueError(
                            "attention_mask penalizes interior window slots; "
                            "this kernel assumes slots 4-7 are mask-free")
                    continue
                c0, c1, mc0 = _slot_cols(k)
                # outside the computed range every valid entry must be masked
                excl = np.ones(QBS, dtype=bool)
                excl[c0:c1] = False
                if (ok[:, None] & m & excl[None, :]).any():
                    raise ValueError(
                        "attention_mask allows entries outside the "
                        "restricted column ranges this kernel computes")
                # inside the range but outside the triangle: no penalties
                inner = ~excl
                inner[mc0:mc0 + 128] = False
                if (pen & inner[None, :]).any():
                    raise ValueError(
                        "attention_mask penalizes inside the restricted "
                        "ranges beyond the 128-wide triangle")
                tri[i, MCOL[k]][pen[:, mc0:mc0 + 128]] = NEG
        # device layout: [128 (ds), NQB, 8, 128 (dt)]
        masktri = np.ascontiguousarray(tri.transpose(2, 0, 1, 3)).astype(NPBF)

        ok_k = (g_k >= 0) & (g_k < T)
        padb = np.ascontiguousarray(np.where(
            ok_k, 0.0, NEG).astype(np.float32).reshape(KB // 128, 128).T)
        in_maps.append(dict(shared, xt=xt, cq=cq, sq=sq, ck=ck, sk=sk,
                            masktri=masktri, padb=padb))
    return in_maps


def _check_mask_coverage(attention_mask):
    """Every True entry for core-c queries must fall inside its 12 slots."""
    am = np.asarray(attention_mask)
    t = np.arange(T)[:, None]
    s = np.arange(T)[None, :]
    h = (t >= QL).astype(np.int64)
    key_start = QL * h - WIN
    i = ((t - QL * h) // QBS)
    lo = key_start + QBS * i
    covered = (s >= lo) & (s < lo + SLOTS * 128)
    if (am & ~covered).any():
        raise ValueError(
            "attention_mask has True entries outside the sliding-window "
            "block structure this kernel is specialized for")


def kernel(x, segment_pos, attention_mask, wq, wk, wv, w_out, b_out):
    x = np.asarray(x, dtype=np.float32)
    wq = np.asarray(wq, dtype=np.float32)
    wk = np.asarray(wk, dtype=np.float32)
    wv = np.asarray(wv, dtype=np.float32)
    w_out = np.asarray(w_out, dtype=np.float32)
    b_out = np.asarray(b_out, dtype=np.float32)

    _check_mask_coverage(attention_mask)

    nc = _get_program()
    shared = {
        "wqt": (np.ascontiguousarray(wq.T) * np.float32(SCALE)).astype(NPBF),
        "wkt": np.ascontiguousarray(wk.T).astype(NPBF),
        "wvt": np.ascontiguousarray(wv.T).astype(NPBF),
        "wot": np.ascontiguousarray(w_out.T).astype(NPBF),
        "bias": b_out,
        "ident": np.eye(128, dtype=np.float32).astype(NPBF),
        "ones": np.ones((128, 1), dtype=np.float32).astype(NPBF),
    }
    in_maps = _prep_core_inputs(x, segment_pos, attention_mask, shared)
    res = run_bass_kernel_spmd(nc, in_maps, list(range(8)))
    global _LAST_RESULT
    _LAST_RESULT = res

    out = np.empty((B, T, W), dtype=np.float32)
    for c in range(8):
        b, h = c // 2, c % 2
        out[b, QL * h:QL * (h + 1), :] = res.results[c]["out"]
    return out


# revision 22
# speedup vs baseline: 1.0059x; 1.0059x over previous
"""Local (sliding-window) MQA attention block on 8 Trainium2 NeuronCores.

Sharding: data-parallel over batch (4) x sequence-parallel over query halves
(2) = 8 cores. Each core computes 1024 query rows of one batch against a
2048-row key halo (window=1024), all 16 query heads, with the single shared
KV head replicated. Outputs are disjoint row-slices of the final projection,
so no cross-core collectives are needed.

v2 layout: all PE operands in bf16 (fast weight load + half the HBM
traffic); x is loaded once (query half kept SBUF-resident and reused by the
q projection); attention S/exp/PV/den restricted to the structurally
non-masked column ranges per key slot with 128x128 triangular masks; the
softmax reciprocal uses the fast approx DVE op; the output projection is
interleaved per query block so the tensor engine absorbs the ACT-bound exp
stretches.

Device kernel phases (per core, identical SPMD program; data differs):
  A1) k/v projection of the query-half keys (also captures x^T into SBUF).
  B)  q projection for all 16 heads from the resident x^T, RoPE fused.
  A0) k/v projection of the remaining halo keys (DMA overlapped with B).
  C)  blocked attention in transposed layout, interleaved per query block
      with D) the output projection + bias.
"""
import sys

for _p in ("/opt/trn_rl_repo",):
    if _p not in sys.path:
        sys.path.insert(0, _p)

import ml_dtypes
import numpy as np

import concourse.bass as bass
import concourse.bacc as bacc
import concourse.tile as tile
import concourse.mybir as mybir
from concourse.bass_utils import run_bass_kernel_spmd

F32 = mybir.dt.float32
BF16 = mybir.dt.bfloat16
NPBF = ml_dtypes.bfloat16
EXP = mybir.ActivationFunctionType.Exp

B, T, W = 4, 2048, 2048
NH, HD = 16, 128
WIN = 1024
QL = 1024          # query rows per core
KB = 2048          # key-halo rows per core
QBS = 512          # query block (moving free dim)
NQB = QL // QBS    # 2 query blocks per core
SLOTS = (WIN + QBS) // 128  # 12 key slots of 128 per query block
NEG = -1.0e9
SCALE = HD ** -0.5
MAX_WAVELENGTH = 10000.0
NW = W // 128      # 16 width chunks

# Attention slot schedule: interior slots first (full query range, so the
# PSUM accumulations for den/PV start with a full write), then the partial
# window-edge (0-3) and causal-diagonal (8-11) slots with restricted column
# ranges. MCOL maps masked slot -> triangle index in the mask tensor.
ORDER = [4, 5, 6, 7, 0, 1, 2, 3, 8, 9, 10, 11]
MCOL = {0: 0, 1: 1, 2: 2, 3: 3, 8: 4, 9: 5, 10: 6, 11: 7}


def _slot_cols(k):
    """(c0, c1, mc0) query-column range computed for slot k and the start of
    its 128-wide mask triangle (None if unmasked)."""
    if 4 <= k <= 7:
        return 0, QBS, None
    if k <= 3:                 # window left edge: cols [0, 128*(k+1))
        return 0, 128 * (k + 1), 128 * k
    c0 = 128 * (k - 8)         # causal diagonal: cols [128*(k-8), QBS)
    return c0, QBS, c0


_COMPILED = None


def _rope_tables(pos):
    """pos: [n] int -> (cmul, smul) [128, n] f32 such that
    rope(x)[d] = x[d]*cmul[d] + x[shuf(d)]*smul[d], shuf(d)=d^32 for d<64."""
    half = 32
    inv_freq = MAX_WAVELENGTH ** (-(2.0 * np.arange(half, dtype=np.float64)) / 64.0)
    ang = pos.astype(np.float64)[None, :] * inv_freq[:, None]   # [32, n]
    sin, cos = np.sin(ang), np.cos(ang)
    n = pos.shape[0]
    cmul = np.ones((HD, n), dtype=np.float64)
    smul = np.zeros((HD, n), dtype=np.float64)
    cmul[0:32] = cos
    cmul[32:64] = cos
    smul[0:32] = -sin
    smul[32:64] = sin
    return cmul.astype(np.float32), smul.astype(np.float32)


def _emit_rope(nc, pool, dst, src_ps, cmul, smul, n):
    """dst[0:64] = src[0:64]*c[0:64] + shuf(src)[0:64]*s[0:64]; dst[64:128]=src.
    dst: SBUF bf16 AP [128, n]; src_ps: PSUM f32 AP [128, n]; cmul/smul f32.
    The partition shuffle is folded into the smul multiplies via offset
    partition APs; only the pass-through copy runs on ScalarE."""
    t1 = pool.tile([64, n], F32, tag="rope_t1", bufs=2)
    t2 = pool.tile([64, n], F32, tag="rope_t2", bufs=2)
    nc.vector.tensor_mul(t1[:, :], src_ps[0:64, :], cmul[0:64, :])
    nc.vector.tensor_mul(t2[0:32, :], src_ps[32:64, :], smul[0:32, :])
    nc.vector.tensor_mul(t2[32:64, :], src_ps[0:32, :], smul[32:64, :])
    nc.vector.tensor_add(dst[0:64, :], t1[:, :], t2[:, :])
    nc.scalar.copy(out=dst[64:128, :], in_=src_ps[64:128, :])


def _build_program():
    nc = bacc.Bacc("TRN2", target_bir_lowering=False, debug=False)

    xt = nc.dram_tensor("xt", [2, W, KB // 2], BF16, kind="ExternalInput")
    wqt = nc.dram_tensor("wqt", [W, W], BF16, kind="ExternalInput")
    wkt = nc.dram_tensor("wkt", [W, HD], BF16, kind="ExternalInput")
    wvt = nc.dram_tensor("wvt", [W, HD], BF16, kind="ExternalInput")
    wot = nc.dram_tensor("wot", [W, W], BF16, kind="ExternalInput")
    bias = nc.dram_tensor("bias", [W], F32, kind="ExternalInput")
    cq_d = nc.dram_tensor("cq", [HD, QL], BF16, kind="ExternalInput")
    sq_d = nc.dram_tensor("sq", [HD, QL], BF16, kind="ExternalInput")
    ck_d = nc.dram_tensor("ck", [HD, KB], BF16, kind="ExternalInput")
    sk_d = nc.dram_tensor("sk", [HD, KB], BF16, kind="ExternalInput")
    masktri_d = nc.dram_tensor("masktri", [128, NQB, 8, 128], BF16,
                               kind="ExternalInput")
    ident_d = nc.dram_tensor("ident", [128, 128], BF16, kind="ExternalInput")
    ones_d = nc.dram_tensor("ones", [128, 1], BF16, kind="ExternalInput")
    padb_d = nc.dram_tensor("padb", [128, KB // 128], F32, kind="ExternalInput")
    out = nc.dram_tensor("out", [QL, W], F32, kind="ExternalOutput")

    with tile.TileContext(nc) as tc:
        with tc.tile_pool(name="persist", bufs=1) as pp:
            kTq = [pp.tile([HD, 512], BF16, tag="kT", bufs=4,
                           name=f"kT{j}") for j in range(4)]  # rope'd k^T
            vq = [pp.tile([128, 512], BF16, tag="v", bufs=4,
                          name=f"v{j}") for j in range(4)]    # natural v
            qT_w = [pp.tile([HD, 4, QL], BF16, tag="qT", bufs=4,
                            name=f"qT{w}") for w in range(4)]
            masks = pp.tile([128, NQB, 8, 128], BF16, tag="masks")
            ident = pp.tile([128, 128], BF16, tag="ident")
            ones_sb = pp.tile([128, 1], BF16, tag="ones")
            bias_bc = pp.tile([128, W], F32, tag="biasbc")
            padb = pp.tile([128, KB // 128], F32, tag="padb")

            # ---------- Phases A/B: projections ----------
            with tc.tile_pool(name="ab", bufs=1) as ab, \
                 tc.tile_pool(name="pb", bufs=3) as pb:
                # resident x^T query rows, one tile per width chunk so each
                # kv/q matmul depends only on its own chunk's DMA
                xq = [ab.tile([128, QL], BF16, tag="xq", bufs=NW,
                              name=f"xq{w}") for w in range(NW)]
                wk_sb = ab.tile([128, NW, HD], BF16, tag="wk")
                wv_sb = ab.tile([128, NW, HD], BF16, tag="wv")
                cq = ab.tile([HD, QL], BF16, tag="cq")
                sq = ab.tile([HD, QL], BF16, tag="sq")
                ck = ab.tile([HD, KB], BF16, tag="ck")
                sk = ab.tile([HD, KB], BF16, tag="sk")
                nc.scalar.dma_start(
                    out=wk_sb[:, :, :],
                    in_=wkt.ap().rearrange("(c p) h -> p c h", p=128))
                nc.scalar.dma_start(
                    out=wv_sb[:, :, :],
                    in_=wvt.ap().rearrange("(c p) h -> p c h", p=128))

                # wq wave tiles; waves 0/1 prefetched before phase A's rope
                # work occupies the scalar queue.
                wq_tiles = [pb.tile([128, NW, 512], BF16, tag="wqw", bufs=2,
                                    name=f"wqw{w}") for w in range(4)]

                def dma_wq(wave):
                    for dm in range(4):
                        nc.scalar.dma_start(
                            out=wq_tiles[wave][:, 4 * dm:4 * (dm + 1), :],
                            in_=wqt[512 * dm:512 * (dm + 1),
                                    128 * 4 * wave:128 * 4 * (wave + 1)]
                            .rearrange("(c p) h -> p c h", p=128))
                nc.gpsimd.dma_start(out=ck[:, :], in_=ck_d[:, :])
                nc.gpsimd.dma_start(out=sk[:, :], in_=sk_d[:, :])
                nc.gpsimd.dma_start(out=ident[:, :], in_=ident_d[:, :])
                nc.gpsimd.dma_start(out=ones_sb[:, :], in_=ones_d[:, :])

                def emit_kv_half(sq2, pa, paps, xt_tile):
                    """kv projection of halo cols [1024*sq2, 1024*(sq2+1)).
                    xt_tile(wc) -> SBUF AP [128, 1024] holding that x^T chunk
                    (also responsible for issuing its DMA)."""
                    kt_ps = [paps.tile([HD, 512], F32, tag="kt_ps",
                                       name="kt_ps") for _ in range(2)]
                    vt_ps = [paps.tile([HD, 512], F32, tag="vt_ps",
                                       name="vt_ps") for _ in range(2)]
                    for wc in range(NW):
                        xch = xt_tile(wc)
                        for hf in range(2):
                            nc.tensor.matmul(
                                out=kt_ps[hf][:, :], lhsT=wk_sb[:, wc, :],
                                rhs=xch[:, QBS * hf:QBS * (hf + 1)],
                                start=(wc == 0), stop=(wc == NW - 1))
                            nc.tensor.matmul(
                                out=vt_ps[hf][:, :], lhsT=wv_sb[:, wc, :],
                                rhs=xch[:, QBS * hf:QBS * (hf + 1)],
                                start=(wc == 0), stop=(wc == NW - 1))
                    for hf in ((1, 0) if sq2 == 0 else (0, 1)):
                        sq4 = 2 * sq2 + hf
                        cols = slice(512 * sq4, 512 * (sq4 + 1))
                        _emit_rope(nc, pa, kTq[sq4][:, :], kt_ps[hf][:, :],
                                   ck[:, cols], sk[:, cols], 512)
                        # v: copy PSUM->SBUF then PE-transpose 128-blocks
                        vt_sb = pa.tile([HD, 512], BF16, tag="vt_sb")
                        nc.vector.tensor_copy(out=vt_sb[:, :],
                                              in_=vt_ps[hf][:, :])
                        for j in range(4):
                            vps2 = paps.tile([128, 128], BF16, tag="vT2")
                            nc.tensor.transpose(
                                vps2[:, :], vt_sb[:, 128 * j:128 * (j + 1)],
                                ident[:, :])
                            nc.vector.tensor_copy(
                                out=vq[sq4][:, 128 * j:128 * (j + 1)],
                                in_=vps2[:, :])

                # ----- A1: query-half keys; captures xq -----
                with tc.tile_pool(name="pa1", bufs=3) as pa, \
                     tc.tile_pool(name="pa1_ps", bufs=2, space="PSUM") as paps:
                    def xq_tile(wc):
                        nc.sync.dma_start(
                            out=xq[wc][:, :],
                            in_=xt[0, 128 * wc:128 * (wc + 1), :])
                        return xq[wc][:, :]
                    emit_kv_half(1, pa, paps, xq_tile)

                dma_wq(0)
                nc.gpsimd.dma_start(out=cq[:, :], in_=cq_d[:, :])
                nc.gpsimd.dma_start(out=sq[:, :], in_=sq_d[:, :])

                # ----- A0: halo keys, streamed right behind the xq DMAs
                # (the x stream saturates HBM while the PE does kv-proj) -----
                with tc.tile_pool(name="pa0", bufs=3) as pa0s, \
                     tc.tile_pool(name="pa0_ps", bufs=2, space="PSUM") as paps0:
                    def xh_tile(wc):
                        t = pa0s.tile([128, 1024], BF16, tag="xh")
                        nc.sync.dma_start(
                            out=t[:, :], in_=xt[1, 128 * wc:128 * (wc + 1), :])
                        return t[:, :]
                    emit_kv_half(0, pa0s, paps0, xh_tile)

                dma_wq(1)
                nc.gpsimd.dma_start(out=padb[:, :], in_=padb_d[:, :])
                nc.gpsimd.dma_start(out=masks[:, :, :, :],
                                    in_=masktri_d[:, :, :, :])
                # ----- B: q projection; waves of [4,4,4,2,2] heads so the
                # final rope tail is short (C's PSUM pools reuse B's banks) -----
                with tc.tile_pool(name="pb_ps", bufs=8, space="PSUM") as pbps:
                    for wave, (h0, nh) in enumerate(
                            [(0, 4), (4, 4), (8, 4), (12, 2), (14, 2)]):
                        q_ps = [[pbps.tile([HD, QBS], F32, tag="q_ps",
                                           name="q_ps")
                                 for _ in range(2)] for _ in range(nh)]
                        for wc in range(NW):
                            for j4 in range(nh):
                                head = h0 + j4
                                wq_w = wq_tiles[head // 4]
                                jc = head % 4
                                for qh in range(2):
                                    nc.tensor.matmul(
                                        out=q_ps[j4][qh][:, :],
                                        lhsT=wq_w[:, wc,
                                                  128 * jc:128 * (jc + 1)],
                                        rhs=xq[wc][:, QBS * qh:QBS * (qh + 1)],
                                        start=(wc == 0),
                                        stop=(wc == NW - 1))
                        if wave < 2:
                            dma_wq(wave + 2)
                        for j4 in range(nh):
                            head = h0 + j4
                            for qh in range(2):
                                _emit_rope(
                                    nc, pb,
                                    qT_w[head // 4][:, head % 4,
                                                    QBS * qh:QBS * (qh + 1)],
                                    q_ps[j4][qh][:, :],
                                    cq[:, QBS * qh:QBS * (qh + 1)],
                                    sq[:, QBS * qh:QBS * (qh + 1)], QBS)


            # ---------- Phases C+D interleaved per query block ----------
            GS = 2  # slots per pipeline group
            NG = SLOTS // GS
            LA = 3  # acc groups trail S/exp groups by LA
            with tc.tile_pool(name="penc", bufs=1) as penc, \
                 tc.tile_pool(name="pd", bufs=2) as pd, \
                 tc.tile_pool(name="pdo", bufs=3) as pdo:
                encU = penc.tile([HD, NH, QL], BF16, tag="encU")
                b_ap = bias.ap()
                nc.gpsimd.dma_start(out=bias_bc[:, :], in_=bass.AP(
                    tensor=b_ap.tensor, offset=b_ap.offset,
                    ap=[[0, 128]] + list(b_ap.ap)))

                def emit_d_tile(i, oc, tsub, wot_sb, dpool):
                    o_ps = dpool.tile([128, 512], F32, tag="o_ps")
                    for n in range(NH):
                        nc.tensor.matmul(
                            out=o_ps[:, :],
                            lhsT=encU[:, n, 128 * tsub:128 * (tsub + 1)],
                            rhs=wot_sb[:, n, :],
                            start=(n == 0), stop=(n == NH - 1))
                    o_sb = pdo.tile([128, 512], F32, tag="o_sb")
                    nc.vector.tensor_add(
                        o_sb[:, :], o_ps[:, :],
                        bias_bc[:, 512 * oc:512 * (oc + 1)])
                    nc.sync.dma_start(
                        out=out[128 * tsub:128 * (tsub + 1),
                                512 * oc:512 * (oc + 1)],
                        in_=o_sb[:, :])

                def dma_wot(oc):
                    wot_sb = pd.tile([128, NW, 512], BF16, tag="wot")
                    for dm in range(4):
                        nc.gpsimd.dma_start(
                            out=wot_sb[:, 4 * dm:4 * (dm + 1), :],
                            in_=wot[512 * dm:512 * (dm + 1),
                                    512 * oc:512 * (oc + 1)]
                            .rearrange("(c p) h -> p c h", p=128))
                    return wot_sb

                CP = {}  # C-phase pools, bound once the inner scope opens

                def emit_head(i, head):
                    pc, pe_t, pcs, pca, pcd = (CP['pc'], CP['et'], CP['pcs'],
                                               CP['pca'], CP['pcd'])
                    enc_ps = pca.tile([HD, QBS], F32, tag="enc_ps")
                    den_ps = pcd.tile([1, QBS], F32, tag="den_ps")
                    ets = [None] * SLOTS
                    eint = [None]
                    qrow = qT_w[head // 4][:, head % 4, :]

                    def emit_s_group(g):
                        sps = []
                        for kk in range(GS):
                            k = ORDER[GS * g + kk]
                            c0, c1, mc0 = _slot_cols(k)
                            nc_ = c1 - c0
                            s_ps = pcs.tile([128, QBS], F32, tag="s_ps")
                            kc = 512 * i + 128 * k
                            nc.tensor.matmul(
                                out=s_ps[:, 0:nc_],
                                lhsT=kTq[kc // 512][:, kc % 512:kc % 512 + 128],
                                rhs=qrow[:, QBS * i + c0:QBS * i + c1],
                                start=True, stop=(mc0 is None))
                            if mc0 is not None:
                                # += triangular mask on PE
                                t0 = mc0 - c0
                                nc.tensor.matmul(
                                    out=s_ps[:, t0:t0 + 128],
                                    lhsT=ident[:, :],
                                    rhs=masks[:, i, MCOL[k], :],
                                    start=False, stop=True)
                            sps.append((s_ps, nc_))
                        for kk in range(GS):
                            k = ORDER[GS * g + kk]
                            blk = 4 * i + k
                            s_ps, nc_ = sps[kk]
                            et = pe_t.tile([128, QBS], BF16, tag="et")
                            nc.scalar.activation(
                                out=et[:, 0:nc_], in_=s_ps[:, 0:nc_],
                                func=EXP,
                                bias=padb[:, blk:blk + 1])
                            ets[k] = et
                        if g == 1:
                            # presum the 4 full interior slots on DVE so den
                            # needs one full-range matmul instead of four
                            e45 = pe_t.tile([128, QBS], BF16, tag="eA")
                            e67 = pe_t.tile([128, QBS], BF16, tag="eB")
                            es = pe_t.tile([128, QBS], BF16, tag="eS")
                            nc.vector.tensor_add(e45[:, :], ets[4][:, :],
                                                 ets[5][:, :])
                            nc.vector.tensor_add(e67[:, :], ets[6][:, :],
                                                 ets[7][:, :])
                            nc.vector.tensor_add(es[:, :], e45[:, :],
                                                 e67[:, :])
                            eint[0] = es

                    def emit_acc_group(g):
                        if g == 1:
                            # den of all four interior slots via the presum
                            nc.tensor.matmul(
                                out=den_ps[:, :], lhsT=ones_sb[:, :],
                                rhs=eint[0][:, :], start=True, stop=False)
                        for kk in range(GS):
                            k = ORDER[GS * g + kk]
                            c0, c1, _ = _slot_cols(k)
                            if not (4 <= k <= 7):
                                nc.tensor.matmul(
                                    out=den_ps[:, c0:c1],
                                    lhsT=ones_sb[:, :],
                                    rhs=ets[k][:, 0:c1 - c0],
                                    start=False,
                                    stop=(g == NG - 1 and kk == GS - 1))
                        for kk in range(GS):
                            k = ORDER[GS * g + kk]
                            c0, c1, _ = _slot_cols(k)
                            blk = 4 * i + k
                            nc.tensor.matmul(
                                out=enc_ps[:, c0:c1],
                                lhsT=vq[blk // 4][:, 128 * (blk % 4):
                                                  128 * (blk % 4 + 1)],
                                rhs=ets[k][:, 0:c1 - c0],
                                start=(g == 0 and kk == 0),
                                stop=(g == NG - 1 and kk == GS - 1))

                    # software pipeline: S groups LA ahead of acc groups
                    for g in range(LA):
                        emit_s_group(g)
                    for g in range(LA, NG):
                        emit_s_group(g)
                        emit_acc_group(g - LA)
                    for g in range(NG - LA, NG):
                        emit_acc_group(g)

                    den_sb = pc.tile([1, QBS], F32, tag="den_sb")
                    nc.vector.reciprocal_approx_fast(
                        out=den_sb[:, :], in_=den_ps[:, :])
                    den_bc = pc.tile([128, QBS], F32, tag="den_bc")
                    nc.gpsimd.partition_broadcast(
                        den_bc[:, :], den_sb[:, :])
                    nc.vector.tensor_mul(
                        encU[:, head, QBS * i:QBS * (i + 1)],
                        enc_ps[:, :], den_bc[:, :])

                with tc.tile_pool(name="pc", bufs=3) as pc, \
                     tc.tile_pool(name="et", bufs=12) as pe_t, \
                     tc.tile_pool(name="pc_s", bufs=4, space="PSUM") as pcs, \
                     tc.tile_pool(name="pc_a", bufs=2, space="PSUM") as pca, \
                     tc.tile_pool(name="pc_d", bufs=1, space="PSUM") as pcd, \
                     tc.tile_pool(name="pd_ps", bufs=1, space="PSUM") as pdps:
                    CP.update(pc=pc, et=pe_t, pcs=pcs, pca=pca, pcd=pcd)
                    # C(i=0)
                    for head in range(NH):
                        emit_head(0, head)
                    # C(i=1) braided with D(i=0): one D tile after each head
                    wot_sb = None
                    for head in range(NH):
                        if head % 4 == 0:
                            wot_sb = dma_wot(head // 4)
                        emit_head(1, head)
                        emit_d_tile(0, head // 4, head % 4, wot_sb, pdps)
                # D(i=1) with its own triple-buffered PSUM accumulators
                with tc.tile_pool(name="pd2_ps", bufs=3, space="PSUM") as pdps2:
                    for oc in range(4):
                        wot_sb = dma_wot(oc)
                        for tsub in range(4, 8):
                            emit_d_tile(1, oc, tsub, wot_sb, pdps2)

    nc.compile()
    return nc


def _get_program():
    global _COMPILED
    if _COMPILED is None:
        _COMPILED = _build_program()
    return _COMPILED


def _prep_core_inputs(x, segment_pos, attention_mask, shared):
    """Per-core input dicts. Core c: batch c//2, query half c%2."""
    segment_pos = np.asarray(segment_pos)
    attention_mask = np.asarray(attention_mask)
    in_maps = []
    for c in range(8):
        b, h = c // 2, c % 2
        key_start = QL * h - WIN
        # halo buffer rows [key_start, key_start + KB) of batch b, zero-padded
        kb = np.zeros((KB, W), dtype=np.float32)
        lo = max(0, -key_start)
        kb[lo:] = x[b, key_start + lo:key_start + KB]
        xtT = kb.T.astype(NPBF)
        xt = np.ascontiguousarray(
            np.stack([xtT[:, WIN:], xtT[:, :WIN]]))  # [2, W, 1024]

        g_q = QL * h + np.arange(QL)                      # global query rows
        g_k = key_start + np.arange(KB)                   # global key rows
        pos_q = segment_pos[g_q]
        pos_k = np.where((g_k >= 0) & (g_k < T), segment_pos[np.clip(g_k, 0, T - 1)], 0)
        cq, sq = (t.astype(NPBF) for t in _rope_tables(pos_q))
        ck, sk = (t.astype(NPBF) for t in _rope_tables(pos_k))

        # Additive mask per (query block i, slot k) in S^T layout [ds, dt],
        # then verified against the restricted-column structure and reduced
        # to 128x128 triangles. Out-of-range (halo padding) rows get NO
        # penalty (excluded via padb in the exp bias; v rows are 0).
        tri = np.zeros((NQB, 8, 128, 128), dtype=np.float32)
        for i in range(NQB):
            t_glob = g_q[QBS * i:QBS * (i + 1)]           # [dt=512]
            for k in range(SLOTS):
                r = QBS * i + 128 * k + np.arange(128)    # halo rows [ds]
                s_glob = key_start + r
                ok = (s_glob >= 0) & (s_glob < T)
                m = attention_mask[t_glob[None, :].repeat(128, 0),
                                   np.clip(s_glob, 0, T - 1)[:, None]]
                pen = ok[:, None] & ~m                    # [128, 512] penalty
                if 4 <= k <= 7:
                    if pen.any():
                        raise ValueError(
                            "attention_mask penalizes interior window slots; "
                            "this kernel assumes slots 4-7 are mask-free")
                    continue
                c0, c1, mc0 = _slot_cols(k)
                # outside the computed range every valid entry must be masked
                excl = np.ones(QBS, dtype=bool)
                excl[c0:c1] = False
                if (ok[:, None] & m & excl[None, :]).any():
                    raise ValueError(
                        "attention_mask allows entries outside the "
                        "restricted column ranges this kernel computes")
                # inside the range but outside the triangle: no penalties
                inner = ~excl
                inner[mc0:mc0 + 128] = False
                if (pen & inner[None, :]).any():
                    raise ValueError(
                        "attention_mask penalizes inside the restricted "
                        "ranges beyond the 128-wide triangle")
                tri[i, MCOL[k]][pen[:, mc0:mc0 + 128]] = NEG
        # device layout: [128 (ds), NQB, 8, 128 (dt)]
        masktri = np.ascontiguousarray(tri.transpose(2, 0, 1, 3)).astype(NPBF)

        ok_k = (g_k >= 0) & (g_k < T)
        padb = np.ascontiguousarray(np.where(
            ok_k, 0.0, NEG).astype(np.float32).reshape(KB // 128, 128).T)
        in_maps.append(dict(shared, xt=xt, cq=cq, sq=sq, ck=ck, sk=sk,
                            masktri=masktri, padb=padb))
    return in_maps


def _check_mask_coverage(attention_mask):
    """Every True entry for core-c queries must fall inside its 12 slots."""
    am = np.asarray(attention_mask)
    t = np.arange(T)[:, None]
    s = np.arange(T)[None, :]
    h = (t >= QL).astype(np.int64)
    key_start = QL * h - WIN
    i = ((t - QL * h) // QBS)
    lo = key_start + QBS * i
    covered = (s >= lo) & (s < lo + SLOTS * 128)
    if (am & ~covered).any():
        raise ValueError(
            "attention_mask has True entries outside the sliding-window "
            "block structure this kernel is specialized for")


def kernel(x, segment_pos, attention_mask, wq, wk, wv, w_out, b_out):
    x = np.asarray(x, dtype=np.float32)
    wq = np.asarray(wq, dtype=np.float32)
    wk = np.asarray(wk, dtype=np.float32)
    wv = np.asarray(wv, dtype=np.float32)
    w_out = np.asarray(w_out, dtype=np.float32)
    b_out = np.asarray(b_out, dtype=np.float32)

    _check_mask_coverage(attention_mask)

    nc = _get_program()
    shared = {
        "wqt": (np.ascontiguousarray(wq.T) * np.float32(SCALE)).astype(NPBF),
        "wkt": np.ascontiguousarray(wk.T).astype(NPBF),
        "wvt": np.ascontiguousarray(wv.T).astype(NPBF),
        "wot": np.ascontiguousarray(w_out.T).astype(NPBF),
        "bias": b_out,
        "ident": np.eye(128, dtype=np.float32).astype(NPBF),
        "ones": np.ones((128, 1), dtype=np.float32).astype(NPBF),
    }
    in_maps = _prep_core_inputs(x, segment_pos, attention_mask, shared)
    res = run_bass_kernel_spmd(nc, in_maps, list(range(8)))
    global _LAST_RESULT
    _LAST_RESULT = res

    out = np.empty((B, T, W), dtype=np.float32)
    for c in range(8):
        b, h = c // 2, c % 2
        out[b, QL * h:QL * (h + 1), :] = res.results[c]["out"]
    return out
